# revision 25
# baseline (speedup 1.0000x reference)
"""MoE layer (8 experts, top-2) on 8 Trainium2 NeuronCores, pair-split
expert-parallel for load balance.

Strategy:
  - Host computes gating + top-2 routing (mirrors the reference ops).
  - Experts are sorted by routed-token count and split into the 4 heaviest
    ("A") and 4 lightest ("B"); pair i = (A_i, B_i) is assigned to the core
    pair (2i, 2i+1). Core 2i holds the FIRST half of the FFN hidden dim
    (f 0:2048) of BOTH its experts, core 2i+1 the second half. Both cores
    process ALL of the pair's tokens, each producing a partial y (its
    hidden-half contribution); the host sums the two partials and adds b2.
  - Per-core work is (capA + capB) * 256 PE cycles instead of
    2 * pad(max_load) * 256 — the padding waste of per-expert capacity is
    replaced by pad(1st) + pad(5th) of the sorted loads, which is nearly
    perfectly balanced. Weight DMA per core is unchanged (half the hidden
    dim of two experts == one full expert); only x and y DMA double.
  - Everything that touches the PE is bfloat16 (same 1 col/cycle rate as
    fp32r, half the HBM traffic, fast-weight-load LDWEIGHTS); PSUM
    accumulation is fp32; rel err ~4e-3 vs the 2e-2 gate.

Device layout: activations are kept transposed ([feature, token]) so both
matmuls consume the natural [K, M] weight layout and the phase-1 output
feeds phase-2 without any on-chip transpose. Per segment the full hidden
half's gelu activations stay SBUF-resident, so phase 2 accumulates all 16
k-tiles in one PSUM group and writes y directly to HBM (no on-chip y
accumulator). Expert weights stream from HBM exactly once.
"""

import numpy as np

N_EMBED = 1024
FFN_HIDDEN = 4096
NUM_EXPERTS = 8
TOP_K = 2
P = 128
KT1 = N_EMBED // P        # 8  k-tiles, phase 1
FH = FFN_HIDDEN // 2      # 2048 hidden per core (half)
MT1 = FH // P             # 16 m-tiles, phase 1 (per half)
KT2 = FH // P             # 16 k-tiles, phase 2 (per half)
MT2 = N_EMBED // P        # 8  m-tiles, phase 2

LAST_RESULT = None        # BassKernelResults of the most recent run


def _ensure_axon_hooks():
    """Make `antenv.axon_hooks` importable so BASS_TRACE=1 degrades
    gracefully instead of crashing when the image lacks the module."""
    try:
        import antenv.axon_hooks  # noqa: F401
        return
    except ImportError:
        pass
    import sys
    import types

    m = types.ModuleType("antenv.axon_hooks")
    m._hook = None
    m.set_axon_ntff_profile_hook = lambda h: setattr(m, "_hook", h)
    m.get_axon_ntff_profile_hook = lambda: m._hook
    sys.modules["antenv.axon_hooks"] = m
    try:
        from trn_agent_boot.trn_boot import _ntff_profile_via_ctypes

        m.set_axon_ntff_profile_hook(_ntff_profile_via_ctypes("/opt/axon/libaxon_pjrt.so"))
    except Exception:
        pass


def _route(x2d, Wg, bg):
    """Top-2 gating. Mirrors the reference (jax softmax + lax.top_k) so the
    selected experts match it exactly; numpy fallback is numerically
    equivalent up to fp32 rounding."""
    try:
        import jax
        import jax.numpy as jnp

        gate = jax.nn.softmax(jnp.asarray(x2d) @ jnp.asarray(Wg) + jnp.asarray(bg), axis=-1)
        scores, idx = jax.lax.top_k(gate, TOP_K)
        scores = np.asarray(scores, dtype=np.float32)
        idx = np.asarray(idx)
    except Exception:
        logits = x2d @ Wg + bg
        m = logits.max(-1, keepdims=True)
        e = np.exp(logits - m)
        p = e / e.sum(-1, keepdims=True)
        order = np.argsort(-p, axis=-1, kind="stable")
        idx = order[:, :TOP_K]
        scores = np.take_along_axis(p, idx, axis=-1)
    scores = scores / (scores.sum(-1, keepdims=True) + 1e-8)
    return idx.astype(np.int64), scores.astype(np.float32)


def _chunks(width, opener=False):
    """Split a token capacity (>=256) into matmul free-dim chunks of at most
    512 columns (PSUM bank limit, fp32), smallest first.  With opener=True a
    128-col chunk leads, so the very first accumulation group only waits on
    a 0.25MB x transfer (used for the segment that gates the kernel start;
    elsewhere it would just add per-matmul overhead)."""
    out, rem = ([128], width - 128) if opener else ([], width)
    while rem > 0:
        if rem <= 512:
            c = rem
        elif rem < 768:
            c = rem - 256
        else:
            c = 512
        out.append(c)
        rem -= c
    if opener:
        out = [out[0]] + sorted(out[1:])
    else:
        out.sort()
    res, off = [], 0
    for c in out:
        res.append((off, c))
        off += c
    return res


def _build_device_program(capA, capB):
    import concourse.tile as tile
    from concourse import bacc, mybir
    from concourse.tile_rust import add_dep_helper

    f32 = mybir.dt.float32
    bf16 = mybir.dt.bfloat16
    gelu = mybir.ActivationFunctionType.Gelu_apprx_tanh
    ident = mybir.ActivationFunctionType.Identity

    nc = bacc.Bacc("TRN2", target_bir_lowering=False, debug=False,
                   num_devices=NUM_EXPERTS)

    segs = [("A", capA), ("B", capB)]
    dram = {}
    for s, cap in segs:
        dram[s] = {
            # [P, KT1, cap]: partition-major so per-k-tile column ranges are
            # single strided DMAs
            "xg": nc.dram_tensor(f"xg{s}", [P, KT1, cap], bf16,
                                 kind="ExternalInput").ap(),
            "w1": nc.dram_tensor(f"w1t{s}", [MT1, P, KT1 * P], bf16,
                                 kind="ExternalInput").ap(),
            "w2": nc.dram_tensor(f"w2t{s}", [MT2, P, KT2 * P], bf16,
                                 kind="ExternalInput").ap(),
            "b1": nc.dram_tensor(f"b1m{s}", [P, MT1], f32,
                                 kind="ExternalInput").ap(),
            "y": nc.dram_tensor(f"yT{s}", [MT2, P, cap], bf16,
                                kind="ExternalOutput").ap(),
        }

    with tile.TileContext(nc) as tc:
        with (
            tc.tile_pool(name="const", bufs=1) as const,
            tc.tile_pool(name="xp", bufs=1) as xp,
            tc.tile_pool(name="hp", bufs=2) as hp,
            tc.tile_pool(name="w1p", bufs=6) as w1p,
            tc.tile_pool(name="w2p", bufs=4) as w2p,
            tc.tile_pool(name="psp", bufs=8, space="PSUM") as psp,
            tc.tile_pool(name="op", bufs=6) as op,
        ):
            chunksA = _chunks(capA, opener=True)
            chunksB = _chunks(capB)

            # prologue: segment A's first chunk gates the first matmul; issue
            # it first (per k-tile, so the transfer spreads across DMA
            # queues), then biases, then the rest of x.  Segment B's x is NOT
            # loaded here — it is emitted on the (otherwise idle) scalar DMA
            # queue after phase 1 A, keeping 2MB out of the congested
            # first ~15us where it caused PE stalls.
            xgA = xp.tile([P, KT1, capA], bf16, name="xgA")
            fcA = chunksA[0][1]
            for kt in range(KT1):
                eng = nc.sync if kt % 2 == 0 else nc.scalar
                eng.dma_start(xgA[:, kt, :fcA], dram["A"]["xg"][:, kt, :fcA])
            b1A = const.tile([P, MT1], f32, name="b1A")
            nc.sync.dma_start(b1A[:], dram["A"]["b1"][:, :])
            b1B = const.tile([P, MT1], f32, name="b1B")
            nc.sync.dma_start(b1B[:], dram["B"]["b1"][:, :])
            # rest of segment A's x, delivered in the order phase 1 consumes
            # it (chunk 1's columns before chunk 2's) and spread across BOTH
            # HWDGE queues (sync + scalar) for early-window bandwidth
            for cs, cw in chunksA[1:]:
                for kt in range(KT1):
                    eng = nc.sync if kt % 2 == 0 else nc.scalar
                    eng.dma_start(xgA[:, kt, cs:cs + cw],
                                  dram["A"]["xg"][:, kt, cs:cs + cw])
            xgB = xp.tile([P, KT1, capB], bf16, name="xgB")

            # HAM warmup: ~3.4us of junk matmuls (into a discarded PSUM slot)
            # run while the prologue DMAs are in flight, so the PE clock-gate
            # has flipped to full rate (2.4GHz) by the time real data lands.
            wjunk = const.tile([P, P], bf16, name="wjunk")
            nc.vector.memset(wjunk[:], 0)
            rjunk = const.tile([P, 512], bf16, name="rjunk")
            nc.vector.memset(rjunk[:], 0)
            wps = psp.tile([P, 512], f32, tag="ps", name="warm")
            NWARM = 8
            for i in range(NWARM):
                nc.tensor.matmul(wps[:], wjunk[:], rjunk[:],
                                 start=(i == 0), stop=(i == NWARM - 1))

            seg_in = {"A": (xgA, b1A, chunksA, capA),
                      "B": (xgB, b1B, chunksB, capB)}

            # anchor instructions used to keep the next weight stream's
            # first DMAs out of the current phase's DMA queues
            anchors = {}

            for si, (s, cap) in enumerate(segs):
                xg_sb, b1_sb, chunks, _ = seg_in[s]
                d = dram[s]

                # ---- phase 1: hT = gelu(W1h.T @ xT + b1h), SBUF-resident
                hT = hp.tile([P, MT1 * cap], bf16, tag="hT", name=f"hT{s}")
                for m in range(MT1):
                    w1m = w1p.tile([P, KT1 * P], bf16, tag="w1")
                    w1dma = nc.gpsimd.dma_start(w1m[:], d["w1"][m, :, :])
                    if s == "B" and m < 2 and "p2A" in anchors:
                        add_dep_helper(w1dma.ins, anchors["p2A"], sync=False,
                                       reason="delay w1B prefetch into phase-2 A")
                    if s == "A" and m in (2, 3) and "p1A0" in anchors:
                        # keep the m2/m3 weight prefetch out of the first ~8us
                        # so the x stream gets the full contended HBM share
                        add_dep_helper(w1dma.ins, anchors["p1A0"], sync=False,
                                       reason="delay w1A m2/m3 behind first act")
                    for ci, (cs, cw) in enumerate(chunks):
                        ps = psp.tile([P, cw], f32, tag="ps", name=f"ps{ci}")
                        for kt in range(KT1):
                            nc.tensor.matmul(
                                ps[:],
                                w1m[:, kt * P:(kt + 1) * P],
                                xg_sb[:, kt, cs:cs + cw],
                                start=(kt == 0),
                                stop=(kt == KT1 - 1),
                            )
                        act = nc.scalar.activation(
                            hT[:, m * cap + cs:m * cap + cs + cw],
                            ps[:],
                            gelu,
                            bias=b1_sb[:, m:m + 1],
                        )
                        if m == 0 and ci == 0:
                            anchors[f"p1{s}0"] = act.ins
                        if m == 4 and ci == 0:
                            anchors[f"p1{s}"] = act.ins

                if s == "A":
                    # segment B's x: on the scalar HWDGE queue (idle), held
                    # past mid-phase-1 so it stays clear of the prologue burst
                    for kt in range(KT1):
                        xbd = nc.scalar.dma_start(xgB[:, kt, :],
                                                  dram["B"]["xg"][:, kt, :])
                        add_dep_helper(xbd.ins, anchors["p1A"], sync=False,
                                       reason="delay xgB load past early phase-1")

                # ---- phase 2: y = W2h.T @ hT (no bias; host adds b2)
                for m in range(MT2):
                    w2m = w2p.tile([P, KT2 * P], bf16, tag="w2")
                    w2dma = nc.gpsimd.dma_start(w2m[:], d["w2"][m, :, :])
                    if m < 2 and f"p1{s}" in anchors:
                        add_dep_helper(w2dma.ins, anchors[f"p1{s}"], sync=False,
                                       reason="delay w2 prefetch past early phase-1")
                    # on the final output tile of the last segment, finish
                    # with the smallest chunk — and split off a 128-col piece
                    # so the kernel tail drains a 32KB store, not 120KB
                    mchunks = chunks
                    if si == len(segs) - 1 and m == MT2 - 1:
                        mchunks = sorted(chunks, key=lambda c: -c[1])
                        cs_l, cw_l = mchunks[-1]
                        if cw_l > 256:
                            mchunks = mchunks[:-1] + [
                                (cs_l, cw_l - 128), (cs_l + cw_l - 128, 128)]
                    for ci, (cs, cw) in enumerate(mchunks):
                        ps = psp.tile([P, cw], f32, tag="ps", name=f"ps{ci}")
                        for kq in range(KT2):
                            nc.tensor.matmul(
                                ps[:],
                                w2m[:, kq * P:(kq + 1) * P],
                                hT[:, kq * cap + cs:cs + kq * cap + cw],
                                start=(kq == 0),
                                stop=(kq == KT2 - 1),
                            )
                        ot = op.tile([P, cw], bf16, tag="o", name=f"o{ci}")
                        act = nc.scalar.activation(ot[:], ps[:], ident)
                        if m == 2 and ci == 0:
                            anchors[f"p2{s}"] = act.ins
                        nc.sync.dma_start(d["y"][m, :, cs:cs + cw], ot[:])

    nc.compile()
    return nc


def kernel(x, Wg, bg, W1, b1, W2, b2):
    global LAST_RESULT
    _ensure_axon_hooks()
    from concourse.bass_utils import run_bass_kernel_spmd
    import ml_dtypes

    bf = ml_dtypes.bfloat16

    x = np.ascontiguousarray(np.asarray(x, dtype=np.float32))
    Wg = np.asarray(Wg, dtype=np.float32)
    bg = np.asarray(bg, dtype=np.float32)
    W1 = np.asarray(W1, dtype=np.float32)
    b1 = np.asarray(b1, dtype=np.float32)
    W2 = np.asarray(W2, dtype=np.float32)
    b2 = np.asarray(b2, dtype=np.float32)

    B, S, D = x.shape
    T = B * S
    xf = x.reshape(T, D)

    top_idx, top_w = _route(xf, Wg, bg)

    tok_idx = []
    tok_w = []
    for e in range(NUM_EXPERTS):
        sel = top_idx == e                       # [T, K]
        rows = np.nonzero(sel.any(axis=1))[0]
        tok_idx.append(rows)
        tok_w.append((top_w * sel).sum(axis=1)[rows].astype(np.float32))

    loads = np.array([len(r) for r in tok_idx])
    order = np.argsort(-loads, kind="stable")
    A_experts = [int(order[i]) for i in range(4)]         # 4 heaviest
    B_experts = [int(order[7 - i]) for i in range(4)]     # paired lightest
    capA = max(256, -(-int(loads[order[0]]) // 16) * 16)
    capB = max(256, -(-int(loads[order[4]]) // 16) * 16)

    nc = _build_device_program(capA, capB)

    def seg_arrays(e, cap, half):
        idx_pad = np.zeros(cap, dtype=np.int64)
        idx_pad[:len(tok_idx[e])] = tok_idx[e]
        # [P, KT1, cap]: partition-major so the device sees one strided DMA
        xg = np.ascontiguousarray(
            xf[idx_pad].T.reshape(KT1, P, cap).transpose(1, 0, 2)).astype(bf)
        w1h = W1[e][:, half * FH:(half + 1) * FH]          # [D, FH]
        w1t = np.ascontiguousarray(
            w1h.reshape(KT1, P, MT1, P).transpose(2, 1, 0, 3)
        ).reshape(MT1, P, KT1 * P).astype(bf)
        w2h = W2[e][half * FH:(half + 1) * FH, :]          # [FH, D]
        w2t = np.ascontiguousarray(
            w2h.reshape(KT2, P, MT2, P).transpose(2, 1, 0, 3)
        ).reshape(MT2, P, KT2 * P).astype(bf)
        b1h = b1[e][half * FH:(half + 1) * FH]
        b1m = np.ascontiguousarray(b1h.reshape(MT1, P).T)
        return xg, w1t, w2t, b1m

    in_maps = []
    for c in range(NUM_EXPERTS):
        pair, half = divmod(c, 2)
        eA, eB = A_experts[pair], B_experts[pair]
        xgA, w1A, w2A, b1A = seg_arrays(eA, capA, half)
        xgB, w1B, w2B, b1B = seg_arrays(eB, capB, half)
        in_maps.append({
            "xgA": xgA, "w1tA": w1A, "w2tA": w2A, "b1mA": b1A,
            "xgB": xgB, "w1tB": w1B, "w2tB": w2B, "b1mB": b1B,
        })

    import os
    trace_cores = None
    if os.environ.get("MOE_TRACE_ALL"):
        trace_cores = list(range(NUM_EXPERTS))
    res = run_bass_kernel_spmd(nc, in_maps, core_ids=list(range(NUM_EXPERTS)),
                               trace_cores=trace_cores)
    LAST_RESULT = res

    out = np.zeros((T, D), dtype=np.float32)
    for pair in range(4):
        c0, c1 = 2 * pair, 2 * pair + 1
        for key, e, cap in (("yTA", A_experts[pair], capA),
                            ("yTB", B_experts[pair], capB)):
            n_e = len(tok_idx[e])
            if n_e == 0:
                continue
            y0 = np.asarray(res.results[c0][key], dtype=np.float32)
            y1 = np.asarray(res.results[c1][key], dtype=np.float32)
            yT = (y0 + y1).reshape(D, cap)[:, :n_e]        # [D, n_e]
            y = yT.T + b2[e][None, :]
            out[tok_idx[e]] += tok_w[e][:, None] * y
    return out.reshape(B, S, D)


# revision 26
# speedup vs baseline: 1.0094x; 1.0094x over previous
"""MoE layer (8 experts, top-2) on 8 Trainium2 NeuronCores, pair-split
expert-parallel for load balance.

Strategy:
  - Host computes gating + top-2 routing (mirrors the reference ops).
  - Experts are sorted by routed-token count and split into the 4 heaviest
    ("A") and 4 lightest ("B"); pair i = (A_i, B_i) is assigned to the core
    pair (2i, 2i+1). Core 2i holds the FIRST half of the FFN hidden dim
    (f 0:2048) of BOTH its experts, core 2i+1 the second half. Both cores
    process ALL of the pair's tokens, each producing a partial y (its
    hidden-half contribution); the host sums the two partials and adds b2.
  - Per-core work is (capA + capB) * 256 PE cycles instead of
    2 * pad(max_load) * 256 — the padding waste of per-expert capacity is
    replaced by pad(1st) + pad(5th) of the sorted loads, which is nearly
    perfectly balanced. Weight DMA per core is unchanged (half the hidden
    dim of two experts == one full expert); only x and y DMA double.
  - Everything that touches the PE is bfloat16 (same 1 col/cycle rate as
    fp32r, half the HBM traffic, fast-weight-load LDWEIGHTS); PSUM
    accumulation is fp32; rel err ~4e-3 vs the 2e-2 gate.

Device layout: activations are kept transposed ([feature, token]) so both
matmuls consume the natural [K, M] weight layout and the phase-1 output
feeds phase-2 without any on-chip transpose. Per segment the full hidden
half's gelu activations stay SBUF-resident, so phase 2 accumulates all 16
k-tiles in one PSUM group and writes y directly to HBM (no on-chip y
accumulator). Expert weights stream from HBM exactly once.
"""

import numpy as np

N_EMBED = 1024
FFN_HIDDEN = 4096
NUM_EXPERTS = 8
TOP_K = 2
P = 128
KT1 = N_EMBED // P        # 8  k-tiles, phase 1
FH = FFN_HIDDEN // 2      # 2048 hidden per core (half)
MT1 = FH // P             # 16 m-tiles, phase 1 (per half)
KT2 = FH // P             # 16 k-tiles, phase 2 (per half)
MT2 = N_EMBED // P        # 8  m-tiles, phase 2

LAST_RESULT = None        # BassKernelResults of the most recent run


def _ensure_axon_hooks():
    """Make `antenv.axon_hooks` importable so BASS_TRACE=1 degrades
    gracefully instead of crashing when the image lacks the module."""
    try:
        import antenv.axon_hooks  # noqa: F401
        return
    except ImportError:
        pass
    import sys
    import types

    m = types.ModuleType("antenv.axon_hooks")
    m._hook = None
    m.set_axon_ntff_profile_hook = lambda h: setattr(m, "_hook", h)
    m.get_axon_ntff_profile_hook = lambda: m._hook
    sys.modules["antenv.axon_hooks"] = m
    try:
        from trn_agent_boot.trn_boot import _ntff_profile_via_ctypes

        m.set_axon_ntff_profile_hook(_ntff_profile_via_ctypes("/opt/axon/libaxon_pjrt.so"))
    except Exception:
        pass


def _route(x2d, Wg, bg):
    """Top-2 gating. Mirrors the reference (jax softmax + lax.top_k) so the
    selected experts match it exactly; numpy fallback is numerically
    equivalent up to fp32 rounding."""
    try:
        import jax
        import jax.numpy as jnp

        gate = jax.nn.softmax(jnp.asarray(x2d) @ jnp.asarray(Wg) + jnp.asarray(bg), axis=-1)
        scores, idx = jax.lax.top_k(gate, TOP_K)
        scores = np.asarray(scores, dtype=np.float32)
        idx = np.asarray(idx)
    except Exception:
        logits = x2d @ Wg + bg
        m = logits.max(-1, keepdims=True)
        e = np.exp(logits - m)
        p = e / e.sum(-1, keepdims=True)
        order = np.argsort(-p, axis=-1, kind="stable")
        idx = order[:, :TOP_K]
        scores = np.take_along_axis(p, idx, axis=-1)
    scores = scores / (scores.sum(-1, keepdims=True) + 1e-8)
    return idx.astype(np.int64), scores.astype(np.float32)


def _chunks(width, opener=False):
    """Split a token capacity (>=256) into matmul free-dim chunks of at most
    512 columns (PSUM bank limit, fp32), smallest first.  With opener=True a
    128-col chunk leads, so the very first accumulation group only waits on
    a 0.25MB x transfer (used for the segment that gates the kernel start;
    elsewhere it would just add per-matmul overhead)."""
    out, rem = ([128], width - 128) if opener else ([], width)
    while rem > 0:
        if rem <= 512:
            c = rem
        elif rem < 768:
            c = rem - 256
        else:
            c = 512
        out.append(c)
        rem -= c
    if opener:
        out = [out[0]] + sorted(out[1:])
    else:
        out.sort()
    res, off = [], 0
    for c in out:
        res.append((off, c))
        off += c
    return res


def _build_device_program(capA, capB):
    import concourse.tile as tile
    from concourse import bacc, mybir
    from concourse.tile_rust import add_dep_helper

    f32 = mybir.dt.float32
    bf16 = mybir.dt.bfloat16
    gelu = mybir.ActivationFunctionType.Gelu_apprx_tanh
    ident = mybir.ActivationFunctionType.Identity

    nc = bacc.Bacc("TRN2", target_bir_lowering=False, debug=False,
                   num_devices=NUM_EXPERTS)

    segs = [("A", capA), ("B", capB)]
    dram = {}
    for s, cap in segs:
        dram[s] = {
            # [P, KT1, cap]: partition-major so per-k-tile column ranges are
            # single strided DMAs
            "xg": nc.dram_tensor(f"xg{s}", [P, KT1, cap], bf16,
                                 kind="ExternalInput").ap(),
            "w1": nc.dram_tensor(f"w1t{s}", [MT1, P, KT1 * P], bf16,
                                 kind="ExternalInput").ap(),
            "w2": nc.dram_tensor(f"w2t{s}", [MT2, P, KT2 * P], bf16,
                                 kind="ExternalInput").ap(),
            "b1": nc.dram_tensor(f"b1m{s}", [P, MT1], f32,
                                 kind="ExternalInput").ap(),
            "y": nc.dram_tensor(f"yT{s}", [MT2, P, cap], bf16,
                                kind="ExternalOutput").ap(),
        }

    with tile.TileContext(nc) as tc:
        with (
            tc.tile_pool(name="const", bufs=1) as const,
            tc.tile_pool(name="xp", bufs=1) as xp,
            tc.tile_pool(name="hp", bufs=2) as hp,
            tc.tile_pool(name="w1p", bufs=4) as w1p,
            tc.tile_pool(name="w2p", bufs=3) as w2p,
            tc.tile_pool(name="psp", bufs=8, space="PSUM") as psp,
            tc.tile_pool(name="op", bufs=6) as op,
        ):
            chunksA = _chunks(capA, opener=True)
            chunksB = _chunks(capB)

            # prologue: segment A's first chunk gates the first matmul; issue
            # it first (per k-tile, so the transfer spreads across DMA
            # queues), then biases, then the rest of x.  Segment B's x is NOT
            # loaded here — it is emitted on the (otherwise idle) scalar DMA
            # queue after phase 1 A, keeping 2MB out of the congested
            # first ~15us where it caused PE stalls.
            xgA = xp.tile([P, KT1, capA], bf16, name="xgA")
            fcA = chunksA[0][1]
            for kt in range(KT1):
                eng = nc.sync if kt % 2 == 0 else nc.scalar
                eng.dma_start(xgA[:, kt, :fcA], dram["A"]["xg"][:, kt, :fcA])
            b1A = const.tile([P, MT1], f32, name="b1A")
            nc.sync.dma_start(b1A[:], dram["A"]["b1"][:, :])
            b1B = const.tile([P, MT1], f32, name="b1B")
            nc.sync.dma_start(b1B[:], dram["B"]["b1"][:, :])
            # rest of segment A's x, delivered in the order phase 1 consumes
            # it (chunk 1's columns before chunk 2's) and spread across BOTH
            # HWDGE queues (sync + scalar) for early-window bandwidth
            for cs, cw in chunksA[1:]:
                for kt in range(KT1):
                    eng = nc.sync if kt % 2 == 0 else nc.scalar
                    eng.dma_start(xgA[:, kt, cs:cs + cw],
                                  dram["A"]["xg"][:, kt, cs:cs + cw])
            xgB = xp.tile([P, KT1, capB], bf16, name="xgB")

            # HAM warmup: ~3.4us of junk matmuls (into a discarded PSUM slot)
            # run while the prologue DMAs are in flight, so the PE clock-gate
            # has flipped to full rate (2.4GHz) by the time real data lands.
            wjunk = const.tile([P, P], bf16, name="wjunk")
            nc.vector.memset(wjunk[:], 0)
            rjunk = const.tile([P, 512], bf16, name="rjunk")
            nc.vector.memset(rjunk[:], 0)
            wps = psp.tile([P, 512], f32, tag="ps", name="warm")
            NWARM = 8
            for i in range(NWARM):
                nc.tensor.matmul(wps[:], wjunk[:], rjunk[:],
                                 start=(i == 0), stop=(i == NWARM - 1))

            seg_in = {"A": (xgA, b1A, chunksA, capA),
                      "B": (xgB, b1B, chunksB, capB)}

            # anchor instructions used to keep the next weight stream's
            # first DMAs out of the current phase's DMA queues
            anchors = {}

            for si, (s, cap) in enumerate(segs):
                xg_sb, b1_sb, chunks, _ = seg_in[s]
                d = dram[s]

                # ---- phase 1: hT = gelu(W1h.T @ xT + b1h), SBUF-resident
                hT = hp.tile([P, MT1 * cap], bf16, tag="hT", name=f"hT{s}")
                for m in range(MT1):
                    w1m = w1p.tile([P, KT1 * P], bf16, tag="w1")
                    w1dma = nc.gpsimd.dma_start(w1m[:], d["w1"][m, :, :])
                    if s == "B" and m < 2 and "p2A" in anchors:
                        add_dep_helper(w1dma.ins, anchors["p2A"], sync=False,
                                       reason="delay w1B prefetch into phase-2 A")
                    if s == "A" and m in (2, 3) and "p1A0" in anchors:
                        # keep the m2/m3 weight prefetch out of the first ~8us
                        # so the x stream gets the full contended HBM share
                        add_dep_helper(w1dma.ins, anchors["p1A0"], sync=False,
                                       reason="delay w1A m2/m3 behind first act")
                    for ci, (cs, cw) in enumerate(chunks):
                        ps = psp.tile([P, cw], f32, tag="ps", name=f"ps{ci}")
                        for kt in range(KT1):
                            nc.tensor.matmul(
                                ps[:],
                                w1m[:, kt * P:(kt + 1) * P],
                                xg_sb[:, kt, cs:cs + cw],
                                start=(kt == 0),
                                stop=(kt == KT1 - 1),
                            )
                        act = nc.scalar.activation(
                            hT[:, m * cap + cs:m * cap + cs + cw],
                            ps[:],
                            gelu,
                            bias=b1_sb[:, m:m + 1],
                        )
                        if m == 0 and ci == 0:
                            anchors[f"p1{s}0"] = act.ins
                        if m == 4 and ci == 0:
                            anchors[f"p1{s}"] = act.ins

                if s == "A":
                    # segment B's x: on the scalar HWDGE queue (idle), held
                    # past mid-phase-1 so it stays clear of the prologue burst
                    for kt in range(KT1):
                        xbd = nc.scalar.dma_start(xgB[:, kt, :],
                                                  dram["B"]["xg"][:, kt, :])
                        add_dep_helper(xbd.ins, anchors["p1A"], sync=False,
                                       reason="delay xgB load past early phase-1")

                # ---- phase 2: y = W2h.T @ hT (no bias; host adds b2)
                for m in range(MT2):
                    w2m = w2p.tile([P, KT2 * P], bf16, tag="w2")
                    w2dma = nc.gpsimd.dma_start(w2m[:], d["w2"][m, :, :])
                    if m < 2 and f"p1{s}" in anchors:
                        add_dep_helper(w2dma.ins, anchors[f"p1{s}"], sync=False,
                                       reason="delay w2 prefetch past early phase-1")
                    # on the final output tile of the last segment, finish
                    # with the smallest chunk — and split off a 128-col piece
                    # so the kernel tail drains a 32KB store, not 120KB
                    mchunks = chunks
                    if si == len(segs) - 1 and m == MT2 - 1:
                        mchunks = sorted(chunks, key=lambda c: -c[1])
                        cs_l, cw_l = mchunks[-1]
                        if cw_l > 256:
                            mchunks = mchunks[:-1] + [
                                (cs_l, cw_l - 128), (cs_l + cw_l - 128, 128)]
                    for ci, (cs, cw) in enumerate(mchunks):
                        ps = psp.tile([P, cw], f32, tag="ps", name=f"ps{ci}")
                        for kq in range(KT2):
                            nc.tensor.matmul(
                                ps[:],
                                w2m[:, kq * P:(kq + 1) * P],
                                hT[:, kq * cap + cs:cs + kq * cap + cw],
                                start=(kq == 0),
                                stop=(kq == KT2 - 1),
                            )
                        ot = op.tile([P, cw], bf16, tag="o", name=f"o{ci}")
                        act = nc.scalar.activation(ot[:], ps[:], ident)
                        if m == 2 and ci == 0:
                            anchors[f"p2{s}"] = act.ins
                        nc.sync.dma_start(d["y"][m, :, cs:cs + cw], ot[:])

    nc.compile()
    return nc


def kernel(x, Wg, bg, W1, b1, W2, b2):
    global LAST_RESULT
    _ensure_axon_hooks()
    from concourse.bass_utils import run_bass_kernel_spmd
    import ml_dtypes

    bf = ml_dtypes.bfloat16

    x = np.ascontiguousarray(np.asarray(x, dtype=np.float32))
    Wg = np.asarray(Wg, dtype=np.float32)
    bg = np.asarray(bg, dtype=np.float32)
    W1 = np.asarray(W1, dtype=np.float32)
    b1 = np.asarray(b1, dtype=np.float32)
    W2 = np.asarray(W2, dtype=np.float32)
    b2 = np.asarray(b2, dtype=np.float32)

    B, S, D = x.shape
    T = B * S
    xf = x.reshape(T, D)

    top_idx, top_w = _route(xf, Wg, bg)

    tok_idx = []
    tok_w = []
    for e in range(NUM_EXPERTS):
        sel = top_idx == e                       # [T, K]
        rows = np.nonzero(sel.any(axis=1))[0]
        tok_idx.append(rows)
        tok_w.append((top_w * sel).sum(axis=1)[rows].astype(np.float32))

    loads = np.array([len(r) for r in tok_idx])
    order = np.argsort(-loads, kind="stable")
    A_experts = [int(order[i]) for i in range(4)]         # 4 heaviest
    B_experts = [int(order[7 - i]) for i in range(4)]     # paired lightest
    capA = max(256, -(-int(loads[order[0]]) // 16) * 16)
    capB = max(256, -(-int(loads[order[4]]) // 16) * 16)

    nc = _build_device_program(capA, capB)

    def seg_arrays(e, cap, half):
        idx_pad = np.zeros(cap, dtype=np.int64)
        idx_pad[:len(tok_idx[e])] = tok_idx[e]
        # [P, KT1, cap]: partition-major so the device sees one strided DMA
        xg = np.ascontiguousarray(
            xf[idx_pad].T.reshape(KT1, P, cap).transpose(1, 0, 2)).astype(bf)
        w1h = W1[e][:, half * FH:(half + 1) * FH]          # [D, FH]
        w1t = np.ascontiguousarray(
            w1h.reshape(KT1, P, MT1, P).transpose(2, 1, 0, 3)
        ).reshape(MT1, P, KT1 * P).astype(bf)
        w2h = W2[e][half * FH:(half + 1) * FH, :]          # [FH, D]
        w2t = np.ascontiguousarray(
            w2h.reshape(KT2, P, MT2, P).transpose(2, 1, 0, 3)
        ).reshape(MT2, P, KT2 * P).astype(bf)
        b1h = b1[e][half * FH:(half + 1) * FH]
        b1m = np.ascontiguousarray(b1h.reshape(MT1, P).T)
        return xg, w1t, w2t, b1m

    in_maps = []
    for c in range(NUM_EXPERTS):
        pair, half = divmod(c, 2)
        eA, eB = A_experts[pair], B_experts[pair]
        xgA, w1A, w2A, b1A = seg_arrays(eA, capA, half)
        xgB, w1B, w2B, b1B = seg_arrays(eB, capB, half)
        in_maps.append({
            "xgA": xgA, "w1tA": w1A, "w2tA": w2A, "b1mA": b1A,
            "xgB": xgB, "w1tB": w1B, "w2tB": w2B, "b1mB": b1B,
        })

    import os
    trace_cores = None
    if os.environ.get("MOE_TRACE_ALL"):
        trace_cores = list(range(NUM_EXPERTS))
    res = run_bass_kernel_spmd(nc, in_maps, core_ids=list(range(NUM_EXPERTS)),
                               trace_cores=trace_cores)
    LAST_RESULT = res

    out = np.zeros((T, D), dtype=np.float32)
    for pair in range(4):
        c0, c1 = 2 * pair, 2 * pair + 1
        for key, e, cap in (("yTA", A_experts[pair], capA),
                            ("yTB", B_experts[pair], capB)):
            n_e = len(tok_idx[e])
            if n_e == 0:
                continue
            y0 = np.asarray(res.results[c0][key], dtype=np.float32)
            y1 = np.asarray(res.results[c1][key], dtype=np.float32)
            yT = (y0 + y1).reshape(D, cap)[:, :n_e]        # [D, n_e]
            y = yT.T + b2[e][None, :]
            out[tok_idx[e]] += tok_w[e][:, None] * y
    return out.reshape(B, S, D)
